# revision 1
# baseline (speedup 1.0000x reference)
"""Trainium2 Bass kernel for nn_MixedLoraModel_734.

Computes, for T=8192 tokens, D=4096:
    out = x @ W_base^T + b_base + scaling[token_lora][:,None] * lora(x)
where lora(x)[t] = WB[l_t] @ (WA[l_t] @ x[t]),  l_t = token_lora[t],
L=8 adapters of rank R=16 (so the full adapter stack is 8*16 = 128 rows).

Strategy (8 NeuronCores, data-parallel over tokens):
  - Each core gets a 1024-token shard of x / token_lora plus replicated
    W_base, b_base, WA, WB, scaling.
  - Routing is done densely with zero data-dependent control flow:
      u_allT[j, t] = sum_d WA_flat[j, d] * x[t, d]          (j = 16*l + r)
      maskT[j, t]  = (j // 16 == token_lora[t])
      u_mT         = u_allT * maskT
      v[t, o]      = sum_j u_mT[j, t] * (scaling[j//16] * WB[j//16, o, j%16])
    Since u_mT is zero outside each token's own adapter block, one dense
    K=128 matmul applies the per-token selected adapter.
  - The base matmul accumulates in PSUM per (token-tile, 256-wide o-chunk);
    the v matmul accumulates into the same PSUM tiles (start=False), then a
    single eviction adds the bias and DMAs out.
  - All matmul operands use float32r (full-rate fp32 PE mode, N>=256).
  - W_base / x / WA / WB are transposed on-chip with PE-transposes (the
    contraction dim must live on SBUF partitions).
"""

import numpy as np

import concourse.bass as bass
import concourse.mybir as mybir
import concourse.tile as tile
from concourse import bacc
from concourse.masks import make_identity

P = 128
D = 4096          # d_in
O = 4096          # d_out
NCORES = 8
T = 8192
TS = T // NCORES  # 1024 tokens per core
NT = TS // P      # 8 token tiles per core
ND = D // P       # 32 contraction chunks
OC = 256          # output-chunk width (PE moving-dim; >=256 keeps f32r at full rate)
NOC = O // OC     # 16
L, R, LR = 8, 16, 128

F32 = mybir.dt.float32
F32R = mybir.dt.float32r
I32 = mybir.dt.int32
EQ = mybir.AluOpType.is_equal
MUL = mybir.AluOpType.mult


def _build() -> bass.Bass:
    nc = bacc.Bacc(None)

    x = nc.declare_dram_parameter("x", [TS, D], F32, isOutput=False)
    w = nc.declare_dram_parameter("w", [O, D], F32, isOutput=False)
    b = nc.declare_dram_parameter("b", [O], F32, isOutput=False)
    wa = nc.declare_dram_parameter("wa", [LR, D], F32, isOutput=False)
    wb = nc.declare_dram_parameter("wb", [L, O, R], F32, isOutput=False)
    scal = nc.declare_dram_parameter("scal", [L], F32, isOutput=False)
    tl = nc.declare_dram_parameter("tl", [TS], I32, isOutput=False)
    out = nc.declare_dram_parameter("out", [TS, O], F32, isOutput=True)

    with tile.TileContext(nc) as tc:
        with (
            tc.tile_pool(name="const", bufs=1) as const,
            tc.tile_pool(name="res", bufs=1) as res,
        ):
            ident = const.tile([P, P], F32)
            make_identity(nc, ident)
            ones_row = const.tile([1, P], F32)
            nc.vector.memset(ones_row[:], 1.0)
            jdiv16 = const.tile([P, 1], F32)

            # Resident operand stacks (all float32r, fed only to the PE):
            # xT:   chunk dc occupies cols [dc*TS, (dc+1)*TS);
            #       xT[p, dc*TS + t] = x[t, dc*128 + p]
            # wbsT: wbsT[j, o] = scaling[j//16] * WB[j//16, o, j%16]
            # u_mT: u_mT[j, t] = masked, per-token-selected  x[t] @ WA[l_t]^T
            xT = res.tile([P, ND * TS], F32R, tag="xT")
            wbsT = res.tile([P, O], F32R, tag="wbsT")
            u_mT = res.tile([P, TS], F32R, tag="u_mT")

            # ---------------- prepass ----------------
            # Phase S: index/scaling columns.
            with (
                tc.tile_pool(name="preS", bufs=1) as preS,
                tc.tile_pool(name="psS", bufs=1, space="PSUM") as psS,
            ):
                scal16 = preS.tile([P, 1], F32, tag="s16")

                irow_i = preS.tile([1, P], I32, tag="iri")
                nc.gpsimd.iota(irow_i[:], pattern=[[1, L], [0, R]], base=0,
                               channel_multiplier=0)
                irow_f = preS.tile([1, P], F32, tag="irf")
                nc.vector.tensor_copy(irow_f[:], irow_i[:])
                pcol = psS.tile([P, 1], F32, tag="pcol")
                nc.tensor.matmul(pcol[:], irow_f[:], ones_row[0:1, 0:1],
                                 start=True, stop=True)
                nc.vector.tensor_copy(jdiv16[:], pcol[:])

                # scal16[p] = scaling[p//16] via E[l, j] = (j//16 == l):
                # scal16 = E^T @ scaling
                scal_sb = preS.tile([L, 1], F32, tag="ssb")
                nc.sync.dma_start(out=scal_sb[:],
                                  in_=scal.rearrange("(p f) -> p f", f=1))
                lcol_i = preS.tile([L, 1], I32, tag="lci")
                nc.gpsimd.iota(lcol_i[:], pattern=[[0, 1]], base=0,
                               channel_multiplier=1)
                lcol_f = preS.tile([L, 1], F32, tag="lcf")
                nc.vector.tensor_copy(lcol_f[:], lcol_i[:])
                ibc8 = psS.tile([L, P], F32, tag="ibc8")
                nc.tensor.matmul(ibc8[:], ones_row[0:1, 0:L], irow_f[:],
                                 start=True, stop=True)
                e_sb = preS.tile([L, P], F32, tag="esb")
                nc.vector.tensor_scalar(e_sb[:], ibc8[:], lcol_f[:], None, EQ)
                s16ps = psS.tile([P, 1], F32, tag="s16ps")
                nc.tensor.matmul(s16ps[:], e_sb[:], scal_sb[:],
                                 start=True, stop=True)
                nc.vector.tensor_copy(scal16[:], s16ps[:])

                # Phase W: adapters. wbsT: per 128-wide o-tile, one DMA gathers
                # [o=128, (l,r)=128], one PE transpose flips to [(l,r), o], and
                # the psum->sbuf eviction folds in scaling while rounding f32r.
                waT = preS.tile([P, D], F32R, tag="waT")
                with (
                    tc.tile_pool(name="preW", bufs=2) as preW,
                    tc.tile_pool(name="psW", bufs=2, space="PSUM") as psW,
                ):
                    for ot in range(O // P):
                        nat = preW.tile([P, P], F32, tag="wbnat")
                        src = wb[:, ot * P:(ot + 1) * P, :].transpose([1, 0, 2])
                        nc.sync.dma_start(out=nat[:], in_=src)
                        pt = psW.tile([P, P], F32, tag="wbps")
                        nc.tensor.transpose(pt[:], nat[:], ident[:])
                        nc.vector.tensor_scalar(wbsT[:, ot * P:(ot + 1) * P],
                                                pt[:], scal16[:], None, MUL)

                    # WA -> WAT chunks [d, j]
                    for q in range(4):
                        wa_nat = preW.tile([P, 1024], F32, tag="nat1k")
                        nc.sync.dma_start(out=wa_nat[:],
                                          in_=wa[:, q * 1024:(q + 1) * 1024])
                        for k in range(8):
                            dc = q * 8 + k
                            pt = psW.tile([P, P], F32, tag="waps")
                            nc.tensor.transpose(pt[:], wa_nat[:, k * P:(k + 1) * P],
                                                ident[:])
                            nc.any.tensor_copy(waT[:, dc * P:(dc + 1) * P], pt[:])

                # Phase X: x -> xT (PE transposes, batched 4 per PSUM bank)
                with (
                    tc.tile_pool(name="preX", bufs=3) as preX,
                    tc.tile_pool(name="psX", bufs=3, space="PSUM") as psX,
                ):
                    for tt in range(NT):
                        for q in range(4):
                            x_nat = preX.tile([P, 1024], F32, tag="nat1k")
                            nc.sync.dma_start(
                                out=x_nat[:],
                                in_=x[tt * P:(tt + 1) * P,
                                      q * 1024:(q + 1) * 1024])
                            for half in range(2):
                                pt = psX.tile([P, 4 * P], F32, tag="xps")
                                for k in range(4):
                                    kk = half * 4 + k
                                    nc.tensor.transpose(
                                        pt[:, k * P:(k + 1) * P],
                                        x_nat[:, kk * P:(kk + 1) * P],
                                        ident[:])
                                for k in range(4):
                                    dc = q * 8 + half * 4 + k
                                    nc.any.tensor_copy(
                                        xT[:, dc * TS + tt * P:
                                           dc * TS + (tt + 1) * P],
                                        pt[:, k * P:(k + 1) * P])

                # Phase U: u_allT + routing mask -> u_mT (two 512-token groups)
                with (
                    tc.tile_pool(name="preU", bufs=2) as preU,
                    tc.tile_pool(name="psU", bufs=2, space="PSUM") as psU,
                ):
                    for g in range(2):
                        t0 = g * 512
                        tli = preU.tile([1, 512], I32, tag="tli")
                        nc.sync.dma_start(
                            out=tli[:],
                            in_=tl[t0:t0 + 512].rearrange("(a f) -> a f", a=1))
                        tlf = preU.tile([1, 512], F32, tag="tlf")
                        nc.vector.tensor_copy(tlf[:], tli[:])
                        tlbc = psU.tile([P, 512], F32, tag="tlbc")
                        nc.tensor.matmul(tlbc[:], ones_row[:], tlf[:],
                                         start=True, stop=True)
                        maskT = preU.tile([P, 512], F32, tag="maskT")
                        nc.vector.tensor_scalar(maskT[:], tlbc[:], jdiv16[:],
                                                None, EQ)
                        ups = psU.tile([P, 512], F32, tag="ups")
                        for dc in range(ND):
                            nc.tensor.matmul(
                                ups[:],
                                waT[:, dc * P:(dc + 1) * P],
                                xT[:, dc * TS + t0: dc * TS + t0 + 512],
                                start=(dc == 0), stop=(dc == ND - 1))
                        nc.vector.tensor_tensor(u_mT[:, t0:t0 + 512], ups[:],
                                                maskT[:], MUL)

            # ---------------- main loop ----------------
            with (
                tc.tile_pool(name="wnat", bufs=5) as wnat_p,
                tc.tile_pool(name="wt", bufs=4) as wt_p,
                tc.tile_pool(name="outp", bufs=4) as out_p,
                tc.tile_pool(name="biasp", bufs=2) as bias_p,
                tc.tile_pool(name="acc_ps", bufs=4, space="PSUM") as acc_ps,
                tc.tile_pool(name="tr_ps", bufs=2, space="PSUM") as tr_ps,
                tc.tile_pool(name="b_ps", bufs=1, space="PSUM") as b_ps,
            ):
                for oc in range(NOC):
                    o0 = oc * OC

                    accs = [acc_ps.tile([P, 512], F32, tag="acc",
                                        name=f"acc{oc}_{g}") for g in range(4)]

                    for dq in range(4):
                        # W rows for this o-chunk / d-quarter: 2 o-subtiles
                        wn = []
                        for os_ in range(2):
                            wtile = wnat_p.tile([P, 1024], F32, tag="wn",
                                                name=f"wn{oc}_{dq}_{os_}")
                            nc.sync.dma_start(
                                out=wtile[:],
                                in_=w[o0 + os_ * P: o0 + (os_ + 1) * P,
                                      dq * 1024:(dq + 1) * 1024])
                            wn.append(wtile)

                        for dr in range(8):
                            dc = dq * 8 + dr
                            pt = tr_ps.tile([P, OC], F32, tag="wtps")
                            for os_ in range(2):
                                nc.tensor.transpose(
                                    pt[:, os_ * P:(os_ + 1) * P],
                                    wn[os_][:, dr * P:(dr + 1) * P],
                                    ident[:])
                            wt = wt_p.tile([P, OC], F32R, tag="wt")
                            nc.any.tensor_copy(wt[:], pt[:])
                            for g in range(4):
                                for h in range(2):
                                    tt = 2 * g + h
                                    nc.tensor.matmul(
                                        accs[g][:, h * OC:(h + 1) * OC],
                                        xT[:, dc * TS + tt * P:
                                           dc * TS + (tt + 1) * P],
                                        wt[:],
                                        start=(dc == 0 and h == 0),
                                        stop=False)

                    # LoRA contribution accumulates into the same PSUM tiles
                    for g in range(4):
                        for h in range(2):
                            tt = 2 * g + h
                            nc.tensor.matmul(
                                accs[g][:, h * OC:(h + 1) * OC],
                                u_mT[:, tt * P:(tt + 1) * P],
                                wbsT[:, o0:o0 + OC],
                                start=False, stop=(h == 1))

                    # bias broadcast for this o-chunk
                    brow = bias_p.tile([1, OC], F32, tag="brow")
                    nc.sync.dma_start(
                        out=brow[:],
                        in_=b[o0:o0 + OC].rearrange("(a f) -> a f", a=1))
                    bps = b_ps.tile([P, OC], F32, tag="bps")
                    nc.tensor.matmul(bps[:], ones_row[:], brow[:],
                                     start=True, stop=True)
                    bias_sb = bias_p.tile([P, OC], F32, tag="bias")
                    nc.any.tensor_copy(bias_sb[:], bps[:])

                    for g in range(4):
                        for h in range(2):
                            tt = 2 * g + h
                            osb = out_p.tile([P, OC], F32, tag="osb",
                                             name=f"osb{oc}_{g}_{h}")
                            nc.vector.tensor_add(
                                osb[:], accs[g][:, h * OC:(h + 1) * OC],
                                bias_sb[:])
                            nc.sync.dma_start(
                                out=out[tt * P:(tt + 1) * P, o0:o0 + OC],
                                in_=osb[:])
    nc.finalize()
    return nc


_NC = None


def _get_nc():
    global _NC
    if _NC is None:
        _NC = _build()
    return _NC


class _Runner:
    """Cached PJRT executable for the SPMD bass kernel.

    Mirrors concourse.bass2jax.run_bass_via_pjrt's multi-core path but
    keeps the jitted shard_map callable alive across invocations so
    repeated kernel() calls skip retrace/recompile.
    """

    # inputs sharded over the token dim; everything else replicated
    _CORE_SHARDED = {"x", "tl"}

    def __init__(self):
        import jax
        import concourse.mybir as mybir_
        from concourse import bass2jax

        bass2jax.install_neuronx_cc_hook()
        self._bass2jax = bass2jax
        nc = _get_nc()
        self.nc = nc

        partition_name = (nc.partition_id_tensor.name
                          if nc.partition_id_tensor else None)
        in_names, out_names, out_avals, zero_outs = [], [], [], []
        for alloc in nc.m.functions[0].allocations:
            if not isinstance(alloc, mybir_.MemoryLocationSet):
                continue
            name = alloc.memorylocations[0].name
            if alloc.kind == "ExternalInput":
                if name != partition_name:
                    in_names.append(name)
            elif alloc.kind == "ExternalOutput":
                shape = tuple(alloc.tensor_shape)
                dtype = mybir_.dt.np(alloc.dtype)
                out_names.append(name)
                out_avals.append(jax.core.ShapedArray(shape, dtype))
                zero_outs.append((shape, dtype))
        self.in_names = list(in_names)
        self.out_names = out_names
        self.out_avals = out_avals
        n_params = len(in_names)
        all_in_names = in_names + out_names
        if partition_name is not None:
            all_in_names.append(partition_name)

        from jax.experimental.shard_map import shard_map
        from jax.sharding import Mesh, NamedSharding, PartitionSpec

        devices = jax.devices()[:NCORES]
        assert len(devices) == NCORES, devices
        mesh = Mesh(np.asarray(devices), ("core",))
        self.mesh = mesh

        def spec_for(name):
            return (PartitionSpec("core") if name in self._CORE_SHARDED
                    else PartitionSpec())

        in_specs = tuple(spec_for(n) for n in in_names) + \
            (PartitionSpec("core"),) * len(out_names)
        out_specs = (PartitionSpec("core"),) * len(out_names)
        self.in_shardings = [NamedSharding(mesh, spec_for(n))
                             for n in in_names]
        self.out_sharding = NamedSharding(mesh, PartitionSpec("core"))

        def _body(*args):
            operands = list(args)
            if partition_name is not None:
                operands.append(bass2jax.partition_id_tensor())
            outs = bass2jax._bass_exec_p.bind(
                *operands,
                out_avals=tuple(out_avals),
                in_names=tuple(all_in_names),
                out_names=tuple(out_names),
                lowering_input_output_aliases=(),
                sim_require_finite=True,
                sim_require_nnan=True,
                nc=nc,
            )
            return tuple(outs)

        self._fn = jax.jit(
            shard_map(_body, mesh=mesh, in_specs=in_specs,
                      out_specs=out_specs, check_rep=False),
            keep_unused=True)
        # resident zero operands for the NEFF's output-tensor inputs (the
        # kernel writes every output element, so contents don't matter and
        # the same device buffers are reused every call)
        import jax
        self._scratch_dev = [
            jax.device_put(
                np.zeros((NCORES * a.shape[0], *a.shape[1:]), a.dtype),
                self.out_sharding)
            for a in out_avals
        ]

    def put_inputs(self, by_name):
        import jax
        out = []
        for name, sharding in zip(self.in_names, self.in_shardings):
            out.append(jax.device_put(by_name[name], sharding))
        return out

    def run_device(self, dev_args):
        """dev_args: device arrays in in_names order. Returns jax arrays."""
        return self._fn(*dev_args, *self._scratch_dev)

    def run(self, by_name):
        outs = self.run_device(self.put_inputs(by_name))
        host = [np.asarray(o) for o in outs]
        return {n: h for n, h in zip(self.out_names, host)}


_RUNNER = None


def _get_runner():
    global _RUNNER
    if _RUNNER is None:
        _RUNNER = _Runner()
    return _RUNNER


def _global_inputs(x, W_base, b_base, WA, WB, scaling, token_lora):
    """Full-size (global) arrays keyed by DRAM-parameter name."""
    return {
        "x": np.ascontiguousarray(np.asarray(x, dtype=np.float32)),
        "w": np.ascontiguousarray(np.asarray(W_base, dtype=np.float32)),
        "b": np.ascontiguousarray(np.asarray(b_base, dtype=np.float32)),
        "wa": np.ascontiguousarray(
            np.asarray(WA, dtype=np.float32).reshape(LR, D)),
        "wb": np.ascontiguousarray(np.asarray(WB, dtype=np.float32)),
        "scal": np.ascontiguousarray(np.asarray(scaling, dtype=np.float32)),
        "tl": np.ascontiguousarray(np.asarray(token_lora, dtype=np.int32)),
    }


def kernel(x, W_base, b_base, WA, WB, scaling, token_lora):
    by_name = _global_inputs(x, W_base, b_base, WA, WB, scaling, token_lora)
    try:
        res = _get_runner().run(by_name)
        return res["out"]
    except Exception:
        # robust fallback through the library SPMD path
        from concourse.bass_utils import run_bass_kernel_spmd

        nc = _get_nc()
        in_maps = []
        for c in range(NCORES):
            in_maps.append({
                "x": by_name["x"][c * TS:(c + 1) * TS],
                "w": by_name["w"],
                "b": by_name["b"],
                "wa": by_name["wa"],
                "wb": by_name["wb"],
                "scal": by_name["scal"],
                "tl": by_name["tl"][c * TS:(c + 1) * TS],
            })
        res = run_bass_kernel_spmd(nc, in_maps, core_ids=list(range(NCORES)))
        return np.concatenate(
            [res.results[c]["out"] for c in range(NCORES)], axis=0)



# revision 7
# speedup vs baseline: 96.7117x; 96.7117x over previous
"""Trainium2 Bass kernel for nn_MixedLoraModel_734.

Computes, for T=8192 tokens, D=4096:
    out = x @ W_base^T + b_base + scaling[token_lora][:,None] * lora(x)
where lora(x)[t] = WB[l_t] @ (WA[l_t] @ x[t]),  l_t = token_lora[t],
L=8 adapters of rank R=16 (the full adapter stack is 8*16 = 128 rows).

Strategy (8 NeuronCores, data-parallel over tokens):
  - Each core receives ONE bf16 blob holding its operands pre-laid-out
    host-side with the contraction dim on partitions:
      xT_sw   x shard transposed, partition-major swizzled
              xT_sw[p, c*TS + t] = x[t, c*128 + p]
      wT      [D, O] W_base transposed (natural [d, o] row-major)
      waT_sw  WA stack transposed, partition-major swizzled
      wbsT    [LR, O]  wbsT[16l+r, o] = scaling[l] * WB[l, o, r]
      mask    [LR, TS] mask[j, t] = (token_lora[t] == j // 16)
      bias    [O], ones [P]
    One packed tensor keeps the per-call PJRT dispatch cost down
    (2 buffer handles instead of 9).
  - Device-side the kernel is pure matmul streaming, no PE transposes:
      u[j, t]  = sum_d waT[d, j] * xT[d, t]       (dense, all adapters)
      u_m      = u * mask                          (per-token selection)
      acc[t,o] = sum_d xT[d, t] * wT[d, o]         (base GEMM, PSUM f32)
               + sum_j u_m[j, t] * wbsT[j, o]      (LoRA, same PSUM)
    eviction adds the (PE-broadcast, f32-resident) bias and DMAs out.
    All matmul operands are bf16 (full PE rate; PSUM accumulates f32;
    abs error ~1e-2 vs the checker's 0.11 tolerance at output scale).
  - DMA instruction count is minimized (HWDGE costs ~625ns per DMA):
    xT in 8 swizzled 8KB-line DMAs, W in 64 three-dim-AP DMAs of
    [128, 8 d-chunks, 512 o], outputs in 64 two-token-tile DMAs.
  - PSUM: 6 banks rotate as [128,512] accumulators (4 per 256-wide
    o-chunk), 2 banks for the u accumulation / bias broadcast.
"""

import numpy as np
import ml_dtypes

import concourse.bass as bass
import concourse.mybir as mybir
import concourse.tile as tile
from concourse import bacc

P = 128
D = 4096          # d_in
O = 4096          # d_out
NCORES = 8
T = 8192
TS = T // NCORES  # 1024 tokens per core
NT = TS // P      # 8 token tiles per core
ND = D // P       # 32 contraction chunks
OC = 256          # output-chunk width
NOC = O // OC     # 16
L, R, LR = 8, 16, 128

F32 = mybir.dt.float32
BF16 = mybir.dt.bfloat16
BF = ml_dtypes.bfloat16
MUL = mybir.AluOpType.mult
ADD = mybir.AluOpType.add

# blob layout (bf16 element offsets)
OFF_XT = 0                      # [P, ND*TS] swizzled
OFF_WT = OFF_XT + D * TS        # [D, O]
OFF_WAT = OFF_WT + D * O        # [P, ND*LR] swizzled
OFF_WBST = OFF_WAT + D * LR     # [LR, O]
OFF_MASK = OFF_WBST + LR * O    # [LR, TS]
OFF_BIAS = OFF_MASK + LR * TS   # [O]
OFF_ONES = OFF_BIAS + O         # [P]
N_BLOB = OFF_ONES + P


def _build() -> bass.Bass:
    nc = bacc.Bacc(None)

    blob = nc.declare_dram_parameter("blob", [N_BLOB], BF16, isOutput=False)
    out = nc.declare_dram_parameter("out", [TS, O], F32, isOutput=True)

    xT_d = blob[OFF_XT:OFF_XT + D * TS].rearrange("(a b) -> a b", b=ND * TS)
    wT_d = blob[OFF_WT:OFF_WT + D * O].rearrange("(a b) -> a b", b=O)
    waT_d = blob[OFF_WAT:OFF_WAT + D * LR].rearrange("(a b) -> a b", b=ND * LR)
    wbsT_d = blob[OFF_WBST:OFF_WBST + LR * O].rearrange("(a b) -> a b", b=O)
    mask_d = blob[OFF_MASK:OFF_MASK + LR * TS].rearrange("(a b) -> a b", b=TS)
    bias_d = blob[OFF_BIAS:OFF_BIAS + O].rearrange("(a b) -> a b", a=1)
    ones_d = blob[OFF_ONES:OFF_ONES + P].rearrange("(a b) -> a b", a=1)

    with tile.TileContext(nc) as tc:
        with (
            tc.tile_pool(name="res", bufs=1) as res,
            tc.tile_pool(name="wtp", bufs=8) as wtp,
            tc.tile_pool(name="outp", bufs=6) as outp,
            tc.tile_pool(name="acc_ps", bufs=6, space="PSUM") as acc_ps,
            tc.tile_pool(name="u_ps", bufs=2, space="PSUM") as u_ps,
        ):
            xTb = res.tile([P, ND * TS], BF16, tag="xTb")
            wbsT = res.tile([P, O], BF16, tag="wbsT")
            waT = res.tile([P, ND * LR], BF16, tag="waT")
            maskB = res.tile([P, TS], BF16, tag="maskB")
            maskF = res.tile([P, TS], F32, tag="maskF")
            u_mT = res.tile([P, TS], BF16, tag="u_mT")
            bias_row = res.tile([1, O], BF16, tag="bias_row")
            ones_col = res.tile([1, P], BF16, tag="ones")
            bias_sb = res.tile([P, O], F32, tag="bias_sb")

            # -------- input DMAs (order = queue order on the SP engine) ----
            nc.sync.dma_start(out=bias_row[:], in_=bias_d)
            nc.sync.dma_start(out=ones_col[:], in_=ones_d)

            # bias broadcast to all 128 partitions, resident f32 (fills the
            # head while the big DMAs stream)
            for bb in range(8):
                bps = u_ps.tile([P, 512], F32, tag="ups", name=f"bias_ps{bb}")
                nc.tensor.matmul(bps[:], ones_col[0:1, :],
                                 bias_row[0:1, bb * 512:(bb + 1) * 512],
                                 start=True, stop=True)
                nc.any.tensor_copy(bias_sb[:, bb * 512:(bb + 1) * 512], bps[:])

            def wt_fetch(ocp, dq):
                """One DMA: d-chunks dq*8..dq*8+7, o = ocp*512..ocp*512+512."""
                wtb = wtp.tile([P, 8 * 512], BF16, tag="wtb",
                               name=f"wtb{ocp}_{dq}")
                src = wT_d[dq * 1024:(dq + 1) * 1024,
                           ocp * 512:(ocp + 1) * 512] \
                    .rearrange("(c p) o -> p c o", p=P)
                dst = wtb[:].rearrange("p (c o) -> p c o", o=512)
                nc.sync.dma_start(out=dst, in_=src)
                return wtb

            # xT eighths (1MB each, 8KB lines) interleaved with the first
            # o-pair's W so the PE can start at the first chunk
            wtb0 = []
            for q in range(8):
                nc.sync.dma_start(
                    out=xTb[:, q * 4 * TS:(q + 1) * 4 * TS],
                    in_=xT_d[:, q * 4 * TS:(q + 1) * 4 * TS])
                if q % 2 == 0:
                    wtb0.append(wt_fetch(0, q // 2))
                if q == 0:
                    nc.sync.dma_start(out=maskB[:], in_=mask_d)
                elif q == 2:
                    nc.sync.dma_start(out=waT[:], in_=waT_d)
            nc.sync.dma_start(out=wbsT[:], in_=wbsT_d)
            nc.vector.tensor_copy(maskF[:], maskB[:])

            ups = [u_ps.tile([P, 512], F32, tag="ups", name=f"ups{g}")
                   for g in range(2)]

            def emit_oc(oc, wtbs, u_after_base=False, out_engines=None):
                o0 = oc * OC
                hx = (oc % 2) * OC
                accs = [acc_ps.tile([P, 512], F32, tag="acc",
                                    name=f"acc{oc}_{g}") for g in range(4)]
                for dc in range(ND):
                    rhs = wtbs[dc // 8][:, (dc % 8) * 512 + hx:
                                        (dc % 8) * 512 + hx + OC]
                    for g in range(4):
                        for h in range(2):
                            tt = 2 * g + h
                            nc.tensor.matmul(
                                accs[g][:, h * OC:(h + 1) * OC],
                                xTb[:, dc * TS + tt * P:dc * TS + (tt + 1) * P],
                                rhs,
                                start=(dc == 0 and h == 0), stop=False)
                if u_after_base:
                    # dense u for all adapters (xTb fully resident by now),
                    # then per-token selection via the routing mask
                    for g2 in range(2):
                        for dc in range(ND):
                            nc.tensor.matmul(
                                ups[g2][:],
                                waT[:, dc * LR:(dc + 1) * LR],
                                xTb[:, dc * TS + g2 * 512:dc * TS + g2 * 512 + 512],
                                start=(dc == 0), stop=(dc == ND - 1))
                        nc.vector.tensor_tensor(
                            u_mT[:, g2 * 512:(g2 + 1) * 512], ups[g2][:],
                            maskF[:, g2 * 512:(g2 + 1) * 512], MUL)
                # LoRA accumulates into the same PSUM banks
                for g in range(4):
                    for h in range(2):
                        tt = 2 * g + h
                        nc.tensor.matmul(
                            accs[g][:, h * OC:(h + 1) * OC],
                            u_mT[:, tt * P:(tt + 1) * P],
                            wbsT[:, o0:o0 + OC],
                            start=False, stop=(h == 1))
                # evict with bias add; one two-token-tile DMA per bank
                for g in range(4):
                    osb = outp.tile([P, 512], F32, tag="osb",
                                    name=f"osb{oc}_{g}")
                    for h in range(2):
                        nc.any.tensor_tensor(
                            osb[:, h * OC:(h + 1) * OC],
                            accs[g][:, h * OC:(h + 1) * OC],
                            bias_sb[:, o0:o0 + OC], ADD)
                    dst = out[2 * g * P:(2 * g + 2) * P, o0:o0 + OC] \
                        .rearrange("(h p) o -> p h o", p=P)
                    src = osb[:].rearrange("p (h o) -> p h o", o=OC)
                    eng = nc.scalar if out_engines is None else out_engines[g]
                    eng.dma_start(out=dst, in_=src)

            # o-pair 0: oc 0 computes u between its base GEMM and its LoRA
            emit_oc(0, wtb0, u_after_base=True)
            emit_oc(1, wtb0)
            for ocp in range(1, 8):
                wtbs = [wt_fetch(ocp, dq) for dq in range(4)]
                emit_oc(2 * ocp, wtbs)
                # last oc: drain outputs over both DMA queues (the SP queue
                # has no W fetches left to block)
                last_engines = ([nc.scalar, nc.sync, nc.scalar, nc.sync]
                                if ocp == 7 else None)
                emit_oc(2 * ocp + 1, wtbs, out_engines=last_engines)

    nc.finalize()
    return nc


_NC = None


def _get_nc():
    global _NC
    if _NC is None:
        _NC = _build()
    return _NC


class _Runner:
    """Cached PJRT executable for the SPMD bass kernel.

    Mirrors concourse.bass2jax.run_bass_via_pjrt's multi-core path but
    keeps the jitted shard_map callable alive across invocations so
    repeated kernel() calls skip retrace/recompile.
    """

    def __init__(self):
        import jax
        import concourse.mybir as mybir_
        from concourse import bass2jax

        bass2jax.install_neuronx_cc_hook()
        self._bass2jax = bass2jax
        nc = _get_nc()
        self.nc = nc

        partition_name = (nc.partition_id_tensor.name
                          if nc.partition_id_tensor else None)
        in_names, out_names, out_avals = [], [], []
        for alloc in nc.m.functions[0].allocations:
            if not isinstance(alloc, mybir_.MemoryLocationSet):
                continue
            name = alloc.memorylocations[0].name
            if alloc.kind == "ExternalInput":
                if name != partition_name:
                    in_names.append(name)
            elif alloc.kind == "ExternalOutput":
                shape = tuple(alloc.tensor_shape)
                dtype = mybir_.dt.np(alloc.dtype)
                out_names.append(name)
                out_avals.append(jax.core.ShapedArray(shape, dtype))
        self.in_names = list(in_names)
        self.out_names = out_names
        self.out_avals = out_avals
        all_in_names = in_names + out_names
        if partition_name is not None:
            all_in_names.append(partition_name)

        from jax.experimental.shard_map import shard_map
        from jax.sharding import Mesh, NamedSharding, PartitionSpec

        devices = jax.devices()[:NCORES]
        assert len(devices) == NCORES, devices
        mesh = Mesh(np.asarray(devices), ("core",))
        self.mesh = mesh

        n_in = len(in_names)
        in_specs = (PartitionSpec("core"),) * (n_in + len(out_names))
        out_specs = (PartitionSpec("core"),) * len(out_names)
        self.out_sharding = NamedSharding(mesh, PartitionSpec("core"))
        self.in_shardings = [self.out_sharding] * n_in

        def _body(*args):
            operands = list(args)
            if partition_name is not None:
                operands.append(bass2jax.partition_id_tensor())
            outs = bass2jax._bass_exec_p.bind(
                *operands,
                out_avals=tuple(out_avals),
                in_names=tuple(all_in_names),
                out_names=tuple(out_names),
                lowering_input_output_aliases=(),
                sim_require_finite=True,
                sim_require_nnan=True,
                nc=nc,
            )
            return tuple(outs)

        self._fn = jax.jit(
            shard_map(_body, mesh=mesh, in_specs=in_specs,
                      out_specs=out_specs, check_rep=False),
            keep_unused=True)
        # resident zero operands for the NEFF's output-tensor inputs (the
        # kernel writes every output element, so contents don't matter and
        # the same device buffers are reused every call)
        self._scratch_dev = [
            jax.device_put(
                np.zeros((NCORES * a.shape[0], *a.shape[1:]), a.dtype),
                self.out_sharding)
            for a in out_avals
        ]

    def put_inputs(self, by_name):
        import jax
        return [jax.device_put(by_name[name], sharding)
                for name, sharding in zip(self.in_names, self.in_shardings)]

    def run_device(self, dev_args):
        """dev_args: device arrays in in_names order. Returns jax arrays."""
        return self._fn(*dev_args, *self._scratch_dev)

    def run(self, by_name):
        outs = self.run_device(self.put_inputs(by_name))
        host = [np.asarray(o) for o in outs]
        return {n: h for n, h in zip(self.out_names, host)}


_RUNNER = None


def _get_runner():
    global _RUNNER
    if _RUNNER is None:
        _RUNNER = _Runner()
    return _RUNNER


def _pack_blobs(x, W_base, b_base, WA, WB, scaling, token_lora):
    """Host-side packing into one bf16 blob per core, concatenated on
    axis 0 (global [NCORES * N_BLOB] for the core-sharded runner)."""
    x = np.asarray(x, np.float32)
    W = np.asarray(W_base, np.float32)
    b = np.asarray(b_base, np.float32)
    WA_ = np.asarray(WA, np.float32)
    WB_ = np.asarray(WB, np.float32)
    sc = np.asarray(scaling, np.float32)
    tl = np.asarray(token_lora, np.int32)

    wT = np.ascontiguousarray(W.T).astype(BF).ravel()
    # waT swizzled: waT_sw[p, c*LR + j] = WA_flat[j, c*128 + p]
    waT = np.ascontiguousarray(
        WA_.reshape(LR, D).T.reshape(ND, P, LR).transpose(1, 0, 2)
        .reshape(P, ND * LR)).astype(BF).ravel()
    wbsT = np.ascontiguousarray(
        (WB_ * sc[:, None, None]).transpose(0, 2, 1).reshape(LR, O)
    ).astype(BF).ravel()
    bias = b.astype(BF)
    ones = np.ones(P, BF)
    jdiv = (np.arange(LR) // R).astype(np.int32)

    blobs = np.empty((NCORES, N_BLOB), BF)
    for c in range(NCORES):
        xs = x[c * TS:(c + 1) * TS]
        tls = tl[c * TS:(c + 1) * TS]
        row = blobs[c]
        # xT swizzled: xT_sw[p, c*TS + t] = x[t, c*128 + p]
        row[OFF_XT:OFF_XT + D * TS] = np.ascontiguousarray(
            xs.T.reshape(ND, P, TS).transpose(1, 0, 2).reshape(P, ND * TS)
        ).astype(BF).ravel()
        row[OFF_WT:OFF_WT + D * O] = wT
        row[OFF_WAT:OFF_WAT + D * LR] = waT
        row[OFF_WBST:OFF_WBST + LR * O] = wbsT
        row[OFF_MASK:OFF_MASK + LR * TS] = \
            (jdiv[:, None] == tls[None, :]).astype(BF).ravel()
        row[OFF_BIAS:OFF_BIAS + O] = bias
        row[OFF_ONES:OFF_ONES + P] = ones
    return blobs


def _global_inputs(x, W_base, b_base, WA, WB, scaling, token_lora):
    """Full-size (global) arrays keyed by DRAM-parameter name."""
    blobs = _pack_blobs(x, W_base, b_base, WA, WB, scaling, token_lora)
    return {"blob": blobs.reshape(NCORES * N_BLOB)}


def kernel(x, W_base, b_base, WA, WB, scaling, token_lora):
    by_name = _global_inputs(x, W_base, b_base, WA, WB, scaling, token_lora)
    try:
        res = _get_runner().run(by_name)
        return res["out"]
    except Exception:
        # robust fallback through the library SPMD path
        from concourse.bass_utils import run_bass_kernel_spmd

        nc = _get_nc()
        blob = by_name["blob"].reshape(NCORES, N_BLOB)
        in_maps = [{"blob": blob[c]} for c in range(NCORES)]
        res = run_bass_kernel_spmd(nc, in_maps, core_ids=list(range(NCORES)))
        return np.concatenate(
            [res.results[c]["out"] for c in range(NCORES)], axis=0)


# revision 9
# speedup vs baseline: 96.7834x; 1.0007x over previous
"""Trainium2 Bass kernel for nn_MixedLoraModel_734.

Computes, for T=8192 tokens, D=4096:
    out = x @ W_base^T + b_base + scaling[token_lora][:,None] * lora(x)
where lora(x)[t] = WB[l_t] @ (WA[l_t] @ x[t]),  l_t = token_lora[t],
L=8 adapters of rank R=16 (the full adapter stack is 8*16 = 128 rows).

Strategy (8 NeuronCores, data-parallel over tokens):
  - Each core receives ONE bf16 blob holding its operands pre-laid-out
    host-side with the contraction dim on partitions:
      xT_sw   x shard transposed, partition-major swizzled
              xT_sw[p, c*TS + t] = x[t, c*128 + p]
      wT      [D, O] W_base transposed (natural [d, o] row-major)
      waT_sw  WA stack transposed, partition-major swizzled
      wbsT    [LR, O]  wbsT[16l+r, o] = scaling[l] * WB[l, o, r]
      mask    [LR, TS] mask[j, t] = (token_lora[t] == j // 16)
      bias    [O], ones [P]
    One packed tensor keeps the per-call PJRT dispatch cost down
    (2 buffer handles instead of 9).
  - Device-side the kernel is pure matmul streaming, no PE transposes:
      u[j, t]  = sum_d waT[d, j] * xT[d, t]       (dense, all adapters)
      u_m      = u * mask                          (per-token selection)
      acc[t,o] = sum_d xT[d, t] * wT[d, o]         (base GEMM, PSUM f32)
               + sum_j u_m[j, t] * wbsT[j, o]      (LoRA, same PSUM)
    eviction adds the (PE-broadcast, f32-resident) bias and DMAs out.
    All matmul operands are bf16 (full PE rate; PSUM accumulates f32;
    abs error ~1e-2 vs the checker's 0.11 tolerance at output scale).
  - DMA instruction count is minimized (HWDGE costs ~625ns per DMA):
    xT in 8 swizzled 8KB-line DMAs, W in 64 three-dim-AP DMAs of
    [128, 8 d-chunks, 512 o], outputs in 64 two-token-tile DMAs.
  - PSUM: 6 banks rotate as [128,512] accumulators (4 per 256-wide
    o-chunk), 2 banks for the u accumulation / bias broadcast.
"""

import numpy as np
import ml_dtypes

import concourse.bass as bass
import concourse.mybir as mybir
import concourse.tile as tile
from concourse import bacc

P = 128
D = 4096          # d_in
O = 4096          # d_out
NCORES = 8
T = 8192
TS = T // NCORES  # 1024 tokens per core
NT = TS // P      # 8 token tiles per core
ND = D // P       # 32 contraction chunks
OC = 256          # output-chunk width
NOC = O // OC     # 16
L, R, LR = 8, 16, 128

F32 = mybir.dt.float32
BF16 = mybir.dt.bfloat16
BF = ml_dtypes.bfloat16
MUL = mybir.AluOpType.mult
ADD = mybir.AluOpType.add

# blob layout (bf16 element offsets)
OFF_XT = 0                      # [P, ND*TS] swizzled
OFF_WT = OFF_XT + D * TS        # [D, O]
OFF_WAT = OFF_WT + D * O        # [P, ND*LR] swizzled
OFF_WBST = OFF_WAT + D * LR     # [LR, O]
OFF_MASK = OFF_WBST + LR * O    # [LR, TS]
OFF_BIAS = OFF_MASK + LR * TS   # [O]
OFF_ONES = OFF_BIAS + O         # [P]
N_BLOB = OFF_ONES + P


def _build() -> bass.Bass:
    nc = bacc.Bacc(None)

    blob = nc.declare_dram_parameter("blob", [N_BLOB], BF16, isOutput=False)
    out = nc.declare_dram_parameter("out", [TS, O], F32, isOutput=True)

    xT_d = blob[OFF_XT:OFF_XT + D * TS].rearrange("(a b) -> a b", b=ND * TS)
    wT_d = blob[OFF_WT:OFF_WT + D * O].rearrange("(a b) -> a b", b=O)
    waT_d = blob[OFF_WAT:OFF_WAT + D * LR].rearrange("(a b) -> a b", b=ND * LR)
    wbsT_d = blob[OFF_WBST:OFF_WBST + LR * O].rearrange("(a b) -> a b", b=O)
    mask_d = blob[OFF_MASK:OFF_MASK + LR * TS].rearrange("(a b) -> a b", b=TS)
    bias_d = blob[OFF_BIAS:OFF_BIAS + O].rearrange("(a b) -> a b", a=1)
    ones_d = blob[OFF_ONES:OFF_ONES + P].rearrange("(a b) -> a b", a=1)

    with tile.TileContext(nc) as tc:
        with (
            tc.tile_pool(name="res", bufs=1) as res,
            tc.tile_pool(name="wtp", bufs=8) as wtp,
            tc.tile_pool(name="outp", bufs=4) as outp,
            tc.tile_pool(name="acc_ps", bufs=6, space="PSUM") as acc_ps,
            tc.tile_pool(name="u_ps", bufs=2, space="PSUM") as u_ps,
        ):
            xTb = res.tile([P, ND * TS], BF16, tag="xTb")
            wbsT = res.tile([P, O], BF16, tag="wbsT")
            waT = res.tile([P, ND * LR], BF16, tag="waT")
            maskB = res.tile([P, TS], BF16, tag="maskB")
            maskF = res.tile([P, TS], F32, tag="maskF")
            u_mT = res.tile([P, TS], BF16, tag="u_mT")
            bias_row = res.tile([1, O], BF16, tag="bias_row")
            ones_col = res.tile([1, P], BF16, tag="ones")
            bias_sb = res.tile([P, O], F32, tag="bias_sb")

            # -------- input DMAs (order = queue order on the SP engine) ----
            nc.sync.dma_start(out=bias_row[:], in_=bias_d)
            nc.sync.dma_start(out=ones_col[:], in_=ones_d)

            # bias broadcast to all 128 partitions, resident f32 (fills the
            # head while the big DMAs stream)
            for bb in range(8):
                bps = u_ps.tile([P, 512], F32, tag="ups", name=f"bias_ps{bb}")
                nc.tensor.matmul(bps[:], ones_col[0:1, :],
                                 bias_row[0:1, bb * 512:(bb + 1) * 512],
                                 start=True, stop=True)
                nc.any.tensor_copy(bias_sb[:, bb * 512:(bb + 1) * 512], bps[:])

            def wt_fetch(ocp, dq):
                """One DMA: d-chunks dq*8..dq*8+7, o = ocp*512..ocp*512+512."""
                wtb = wtp.tile([P, 8 * 512], BF16, tag="wtb",
                               name=f"wtb{ocp}_{dq}")
                src = wT_d[dq * 1024:(dq + 1) * 1024,
                           ocp * 512:(ocp + 1) * 512] \
                    .rearrange("(c p) o -> p c o", p=P)
                dst = wtb[:].rearrange("p (c o) -> p c o", o=512)
                nc.sync.dma_start(out=dst, in_=src)
                return wtb

            # xT eighths (1MB each, 8KB lines) interleaved with the first
            # o-pair's W so the PE can start at the first chunk
            wtb0 = []
            for q in range(8):
                nc.sync.dma_start(
                    out=xTb[:, q * 4 * TS:(q + 1) * 4 * TS],
                    in_=xT_d[:, q * 4 * TS:(q + 1) * 4 * TS])
                if q % 2 == 0:
                    wtb0.append(wt_fetch(0, q // 2))
                if q == 0:
                    nc.sync.dma_start(out=maskB[:], in_=mask_d)
                elif q == 2:
                    nc.sync.dma_start(out=waT[:], in_=waT_d)
            nc.sync.dma_start(out=wbsT[:], in_=wbsT_d)
            nc.vector.tensor_copy(maskF[:], maskB[:])

            ups = [u_ps.tile([P, 512], F32, tag="ups", name=f"ups{g}")
                   for g in range(2)]

            def emit_half(ocp, tg, wtbs, fuse_u=False, out_engines=None):
                """One 512-wide o-chunk for token tiles tg*4..tg*4+3.

                512-wide moving operands mean each PE stationary load is
                amortized over 512 cycles (the cost model ignores LdWeights
                but hardware does not), and each matmul fills exactly one
                PSUM bank.
                """
                o0 = ocp * 512
                accs = [acc_ps.tile([P, 512], F32, tag="acc",
                                    name=f"acc{ocp}_{tg}_{i}") for i in range(4)]
                for dc in range(ND):
                    rhs = wtbs[dc // 8][:, (dc % 8) * 512:(dc % 8) * 512 + 512]
                    for i in range(4):
                        tt = tg * 4 + i
                        nc.tensor.matmul(
                            accs[i][:],
                            xTb[:, dc * TS + tt * P:dc * TS + (tt + 1) * P],
                            rhs,
                            start=(dc == 0), stop=False)
                    if fuse_u:
                        # dense u for all adapters rides this dc sweep; the
                        # routing mask selects per-token rows afterwards
                        for g2 in range(2):
                            nc.tensor.matmul(
                                ups[g2][:],
                                waT[:, dc * LR:(dc + 1) * LR],
                                xTb[:, dc * TS + g2 * 512:dc * TS + g2 * 512 + 512],
                                start=(dc == 0), stop=(dc == ND - 1))
                if fuse_u:
                    for g2 in range(2):
                        nc.vector.tensor_tensor(
                            u_mT[:, g2 * 512:(g2 + 1) * 512], ups[g2][:],
                            maskF[:, g2 * 512:(g2 + 1) * 512], MUL)
                # LoRA accumulates into the same PSUM banks
                for i in range(4):
                    tt = tg * 4 + i
                    nc.tensor.matmul(
                        accs[i][:],
                        u_mT[:, tt * P:(tt + 1) * P],
                        wbsT[:, o0:o0 + 512],
                        start=False, stop=True)
                # evict with bias add; one two-token-tile DMA per osb
                for j in range(2):
                    osb = outp.tile([P, 1024], F32, tag="osb",
                                    name=f"osb{ocp}_{tg}_{j}")
                    for i in (2 * j, 2 * j + 1):
                        nc.any.tensor_tensor(
                            osb[:, (i % 2) * 512:(i % 2) * 512 + 512],
                            accs[i][:], bias_sb[:, o0:o0 + 512], ADD)
                    t0 = (tg * 4 + 2 * j) * P
                    dst = out[t0:t0 + 2 * P, o0:o0 + 512] \
                        .rearrange("(h p) o -> p h o", p=P)
                    src = osb[:].rearrange("p (h o) -> p h o", o=512)
                    eng = nc.scalar if out_engines is None else out_engines[j]
                    eng.dma_start(out=dst, in_=src)

            # first o-chunk's first token group carries the fused u sweep
            emit_half(0, 0, wtb0, fuse_u=True)
            emit_half(0, 1, wtb0)
            for ocp in range(1, 8):
                wtbs = [wt_fetch(ocp, dq) for dq in range(4)]
                emit_half(ocp, 0, wtbs)
                # last chunk: drain outputs over both DMA queues (the SP
                # queue has no W fetches left to block)
                last_engines = [nc.scalar, nc.sync] if ocp == 7 else None
                emit_half(ocp, 1, wtbs, out_engines=last_engines)

    nc.finalize()
    return nc


_NC = None


def _get_nc():
    global _NC
    if _NC is None:
        _NC = _build()
    return _NC


class _Runner:
    """Cached PJRT executable for the SPMD bass kernel.

    Mirrors concourse.bass2jax.run_bass_via_pjrt's multi-core path but
    keeps the jitted shard_map callable alive across invocations so
    repeated kernel() calls skip retrace/recompile.
    """

    def __init__(self):
        import jax
        import concourse.mybir as mybir_
        from concourse import bass2jax

        bass2jax.install_neuronx_cc_hook()
        self._bass2jax = bass2jax
        nc = _get_nc()
        self.nc = nc

        partition_name = (nc.partition_id_tensor.name
                          if nc.partition_id_tensor else None)
        in_names, out_names, out_avals = [], [], []
        for alloc in nc.m.functions[0].allocations:
            if not isinstance(alloc, mybir_.MemoryLocationSet):
                continue
            name = alloc.memorylocations[0].name
            if alloc.kind == "ExternalInput":
                if name != partition_name:
                    in_names.append(name)
            elif alloc.kind == "ExternalOutput":
                shape = tuple(alloc.tensor_shape)
                dtype = mybir_.dt.np(alloc.dtype)
                out_names.append(name)
                out_avals.append(jax.core.ShapedArray(shape, dtype))
        self.in_names = list(in_names)
        self.out_names = out_names
        self.out_avals = out_avals
        all_in_names = in_names + out_names
        if partition_name is not None:
            all_in_names.append(partition_name)

        from jax.experimental.shard_map import shard_map
        from jax.sharding import Mesh, NamedSharding, PartitionSpec

        devices = jax.devices()[:NCORES]
        assert len(devices) == NCORES, devices
        mesh = Mesh(np.asarray(devices), ("core",))
        self.mesh = mesh

        n_in = len(in_names)
        in_specs = (PartitionSpec("core"),) * (n_in + len(out_names))
        out_specs = (PartitionSpec("core"),) * len(out_names)
        self.out_sharding = NamedSharding(mesh, PartitionSpec("core"))
        self.in_shardings = [self.out_sharding] * n_in

        def _body(*args):
            operands = list(args)
            if partition_name is not None:
                operands.append(bass2jax.partition_id_tensor())
            outs = bass2jax._bass_exec_p.bind(
                *operands,
                out_avals=tuple(out_avals),
                in_names=tuple(all_in_names),
                out_names=tuple(out_names),
                lowering_input_output_aliases=(),
                sim_require_finite=True,
                sim_require_nnan=True,
                nc=nc,
            )
            return tuple(outs)

        self._fn = jax.jit(
            shard_map(_body, mesh=mesh, in_specs=in_specs,
                      out_specs=out_specs, check_rep=False),
            keep_unused=True)
        # resident zero operands for the NEFF's output-tensor inputs (the
        # kernel writes every output element, so contents don't matter and
        # the same device buffers are reused every call)
        self._scratch_dev = [
            jax.device_put(
                np.zeros((NCORES * a.shape[0], *a.shape[1:]), a.dtype),
                self.out_sharding)
            for a in out_avals
        ]

    def put_inputs(self, by_name):
        import jax
        return [jax.device_put(by_name[name], sharding)
                for name, sharding in zip(self.in_names, self.in_shardings)]

    def run_device(self, dev_args):
        """dev_args: device arrays in in_names order. Returns jax arrays."""
        return self._fn(*dev_args, *self._scratch_dev)

    def run(self, by_name):
        outs = self.run_device(self.put_inputs(by_name))
        host = [np.asarray(o) for o in outs]
        return {n: h for n, h in zip(self.out_names, host)}


_RUNNER = None


def _get_runner():
    global _RUNNER
    if _RUNNER is None:
        _RUNNER = _Runner()
    return _RUNNER


def _pack_blobs(x, W_base, b_base, WA, WB, scaling, token_lora):
    """Host-side packing into one bf16 blob per core, concatenated on
    axis 0 (global [NCORES * N_BLOB] for the core-sharded runner)."""
    x = np.asarray(x, np.float32)
    W = np.asarray(W_base, np.float32)
    b = np.asarray(b_base, np.float32)
    WA_ = np.asarray(WA, np.float32)
    WB_ = np.asarray(WB, np.float32)
    sc = np.asarray(scaling, np.float32)
    tl = np.asarray(token_lora, np.int32)

    wT = np.ascontiguousarray(W.T).astype(BF).ravel()
    # waT swizzled: waT_sw[p, c*LR + j] = WA_flat[j, c*128 + p]
    waT = np.ascontiguousarray(
        WA_.reshape(LR, D).T.reshape(ND, P, LR).transpose(1, 0, 2)
        .reshape(P, ND * LR)).astype(BF).ravel()
    wbsT = np.ascontiguousarray(
        (WB_ * sc[:, None, None]).transpose(0, 2, 1).reshape(LR, O)
    ).astype(BF).ravel()
    bias = b.astype(BF)
    ones = np.ones(P, BF)
    jdiv = (np.arange(LR) // R).astype(np.int32)

    blobs = np.empty((NCORES, N_BLOB), BF)
    for c in range(NCORES):
        xs = x[c * TS:(c + 1) * TS]
        tls = tl[c * TS:(c + 1) * TS]
        row = blobs[c]
        # xT swizzled: xT_sw[p, c*TS + t] = x[t, c*128 + p]
        row[OFF_XT:OFF_XT + D * TS] = np.ascontiguousarray(
            xs.T.reshape(ND, P, TS).transpose(1, 0, 2).reshape(P, ND * TS)
        ).astype(BF).ravel()
        row[OFF_WT:OFF_WT + D * O] = wT
        row[OFF_WAT:OFF_WAT + D * LR] = waT
        row[OFF_WBST:OFF_WBST + LR * O] = wbsT
        row[OFF_MASK:OFF_MASK + LR * TS] = \
            (jdiv[:, None] == tls[None, :]).astype(BF).ravel()
        row[OFF_BIAS:OFF_BIAS + O] = bias
        row[OFF_ONES:OFF_ONES + P] = ones
    return blobs


def _global_inputs(x, W_base, b_base, WA, WB, scaling, token_lora):
    """Full-size (global) arrays keyed by DRAM-parameter name."""
    blobs = _pack_blobs(x, W_base, b_base, WA, WB, scaling, token_lora)
    return {"blob": blobs.reshape(NCORES * N_BLOB)}


def kernel(x, W_base, b_base, WA, WB, scaling, token_lora):
    by_name = _global_inputs(x, W_base, b_base, WA, WB, scaling, token_lora)
    try:
        res = _get_runner().run(by_name)
        return res["out"]
    except Exception:
        # robust fallback through the library SPMD path
        from concourse.bass_utils import run_bass_kernel_spmd

        nc = _get_nc()
        blob = by_name["blob"].reshape(NCORES, N_BLOB)
        in_maps = [{"blob": blob[c]} for c in range(NCORES)]
        res = run_bass_kernel_spmd(nc, in_maps, core_ids=list(range(NCORES)))
        return np.concatenate(
            [res.results[c]["out"] for c in range(NCORES)], axis=0)
